# revision 31
# baseline (speedup 1.0000x reference)
"""Trainium2 Bass kernel for GaussianFlowOccRasterizer bilinear point sampling.

values [2,3,6,256,704,17] f32, indices [500000,3] i32, coors [500000,2] f32
-> out [500000,17] f32 (per-point bilinear sample of image flat(b,t,n) at
pixel (coors - 0.5), zero padding outside).

Strategy (8 NeuronCores):
  - Host re-lays values into "banded pixel-pair" tables: copy A holds row
    pairs (2b,2b+1), copy B holds (2b+1,2b+2); each pixel padded 17->32 f32
    so a band entry (band,x) is 256B = [row_top pix | row_bot pix]. One
    512B dma_gather descriptor starting at entry (band,x0) then fetches all
    4 bilinear corners of a point.
  - 144 band-units (36 img x 2 copies x 2 half-images) are dealt
    round-robin to the 8 cores (uniform 207MB table / core). Points are
    routed to the owning core and sorted by 32768-entry windows so the
    int16 dma_gather indices are window-relative (sharding hint: "route
    points to the owning device by flat index").
  - Device: dma_gather spread over all 4 SWDGE queues (disjoint Q7 cpu
    pairs -> 4x parallel descriptor generation), IRAM pre-warmed with
    dummy gathers, 3-op strided DVE blend, chunked output DMA. Trailing
    window-padding indices are negative so the Q7 ucode trims them.
"""
import os

import numpy as np

_WARM = os.environ.get("KERNEL_WARM", "1") == "1"
_BLEND3 = os.environ.get("KERNEL_BLEND3", "1") == "1"
_NEGPAD = os.environ.get("KERNEL_NEGPAD", "0") == "1"  # crashes HW: keep 0
_BATCH = int(os.environ.get("KERNEL_BATCH", "3"))  # gather calls per blend
_GBUFS = int(os.environ.get("KERNEL_GBUFS", "3"))
_PSUM = os.environ.get("KERNEL_PSUM", "1") == "1"  # blend tiles in PSUM
_PREP = os.environ.get("KERNEL_PREP", "0") == "1"  # broken on HW: keep 0

B, T, N, H, W, C = 2, 3, 6, 256, 704, 17
NIMG = B * T * N
NCORES = 8
CP = 32               # padded channels (128 B / pixel)
ELEM = 4 * CP         # gathered f32 per descriptor (2 entries = 4 pixels)
ENTRY = 2 * CP        # f32 per table entry (pixel pair of one x)
BANDS = H // 2        # 128 bands per copy
UNIT_BANDS = BANDS // 2  # 64 bands per unit (half image per copy)
NUNITS = NIMG * 2 * 2    # 144 units
UNITS_PER_CORE = NUNITS // NCORES  # 18
ENTRIES_PER_UNIT = UNIT_BANDS * W  # 45056
ECORE = UNITS_PER_CORE * ENTRIES_PER_UNIT  # 811008 entries per core
WINDOW = 32768
NWIN = -(-ECORE // WINDOW)  # 25
# 1024 idx = 64 descs/SDMA-engine = the single_packet HW ceiling; more crashes
MAX_CALL = int(os.environ.get("KERNEL_MAXCALL", "1024"))
NQ = 4                # SWDGE queues
OUT_CHUNKS = int(os.environ.get("KERNEL_CHUNKS", "8"))

_cache = {}


def _build_program(schedule):
    """schedule: tuple of (window_id, num_idxs) per dma_gather call,
    num_idxs % 128 == 0, <= MAX_CALL. Same program for all cores."""
    import concourse.bacc as bacc
    import concourse.bass as bass
    import concourse.mybir as mybir
    from concourse import library_config
    from concourse.tile import TileContext

    tot_idx = sum(n for _, n in schedule)
    slots = tot_idx // 128

    nc = bacc.Bacc("TRN2", target_bir_lowering=False, debug=False,
                   num_devices=NCORES, num_swdge_queues=NQ)
    table = nc.declare_dram_parameter(
        "table", [ECORE + 2, ENTRY], mybir.dt.float32, isOutput=False)
    idxs = nc.declare_dram_parameter(
        "idxs", [128, tot_idx // 16], mybir.dt.int16, isOutput=False)
    coors = nc.declare_dram_parameter(
        "coors", [128, slots * 2], mybir.dt.float32, isOutput=False)
    out = nc.declare_dram_parameter(
        "out", [128, slots * C], mybir.dt.float32, isOutput=True)

    f32 = mybir.dt.float32
    Alu = mybir.AluOpType

    with TileContext(nc) as tc:
        with tc.tile_pool(name="io", bufs=1) as io, \
             tc.tile_pool(name="gp", bufs=1) as gp, \
             tc.psum_pool(name="pp", bufs=1) as pp:
            nc.gpsimd.load_library(library_config.mlp)

            # inputs
            idx_t = io.tile([128, tot_idx // 16], mybir.dt.int16)
            nc.sync.dma_start(out=idx_t[:], in_=idxs[:])
            co_t = io.tile([128, slots, 2], f32)
            nc.sync.dma_start(
                out=co_t[:, :, :],
                in_=coors[:].rearrange("p (s c) -> p s c", c=2))
            out_t = io.tile([128, slots, C], f32)
            # wq in PSUM: the blend mults then read one SBUF (gathered g)
            # and one PSUM (weights) operand -> no shared-SBUF-port lock.
            wq = (pp if _PSUM else io).tile([128, slots, 4], f32)

            # IRAM warm-up: one dummy gather per queue (each queue runs on
            # its own Q7 cpu pair; first use pays a ~9-13us code load and
            # the loads SERIALIZE on a shared loader). The warms are
            # interleaved with real gathers on already-warm queues (see
            # the ramp in the main loop). memset on gpsimd so warm q0
            # doesn't wait for the DVE preamble.
            warm_state = {}
            if _WARM:
                wi = io.tile([128, 8], mybir.dt.int16)
                nc.gpsimd.memset(wi[:, :], 0)
                warm_ap = bass.AP(table, 0, [(ENTRY, 2), (1, ELEM)])

                def issue_warm(q):
                    wg = io.tile([128, 1, ELEM], f32, name=f"wg{q}",
                                 uniquify=True)
                    nc.gpsimd.dma_gather(
                        out_ap=wg[:, :, :], in_ap=warm_ap, idxs_ap=wi[:, :],
                        num_idxs=128, num_idxs_reg=128, elem_size=ELEM,
                        elem_step=ENTRY, queue_num=q)

                issue_warm(0)
                warm_state = {4: 1, 10: 2, 19: 3}  # call idx -> warm queue

            # ---- bilinear weights from coors -> wq[128, slots, 4] ----
            with tc.tile_pool(name="wp", bufs=1) as wp:
                def floor_of(src_ap, name):
                    ti = wp.tile([128, slots], mybir.dt.int32, name=f"{name}_i")
                    nc.vector.tensor_copy(out=ti[:], in_=src_ap)
                    tf = wp.tile([128, slots], f32, name=f"{name}_f")
                    nc.vector.tensor_copy(out=tf[:], in_=ti[:])
                    gt = wp.tile([128, slots], f32, name=f"{name}_g")
                    nc.vector.tensor_tensor(out=gt[:], in0=tf[:], in1=src_ap,
                                            op=Alu.is_gt)
                    fl = wp.tile([128, slots], f32, name=f"{name}_fl")
                    nc.vector.tensor_sub(out=fl[:], in0=tf[:], in1=gt[:])
                    return fl

                ix = wp.tile([128, slots], f32)
                nc.vector.tensor_scalar(out=ix[:], in0=co_t[:, :, 1],
                                        scalar1=0.5, scalar2=None,
                                        op0=Alu.subtract)
                iy = wp.tile([128, slots], f32)
                nc.vector.tensor_scalar(out=iy[:], in0=co_t[:, :, 0],
                                        scalar1=0.5, scalar2=None,
                                        op0=Alu.subtract)
                x0 = floor_of(ix[:], "x0")
                y0 = floor_of(iy[:], "y0")
                wx = wp.tile([128, slots], f32)
                nc.vector.tensor_sub(out=wx[:], in0=ix[:], in1=x0[:])
                wy = wp.tile([128, slots], f32)
                nc.vector.tensor_sub(out=wy[:], in0=iy[:], in1=y0[:])

                s = wp.tile([128, slots], f32)      # x0 < 0 (left shift)
                nc.vector.tensor_scalar(out=s[:], in0=x0[:], scalar1=0.0,
                                        scalar2=None, op0=Alu.is_lt)
                sy = wp.tile([128, slots], f32)     # y0 < 0 (top shift)
                nc.vector.tensor_scalar(out=sy[:], in0=y0[:], scalar1=0.0,
                                        scalar2=None, op0=Alu.is_lt)
                # wL = s ? wx : 1-wx ;  wR = (s | x0>=W-1) ? 0 : wx
                one_m_wx = wp.tile([128, slots], f32)
                nc.vector.tensor_scalar(out=one_m_wx[:], in0=wx[:],
                                        scalar1=-1.0, scalar2=1.0,
                                        op0=Alu.mult, op1=Alu.add)
                d = wp.tile([128, slots], f32)
                nc.vector.tensor_sub(out=d[:], in0=wx[:], in1=one_m_wx[:])
                wL = wp.tile([128, slots], f32)
                nc.vector.tensor_mul(out=wL[:], in0=s[:], in1=d[:])
                nc.vector.tensor_add(out=wL[:], in0=wL[:], in1=one_m_wx[:])
                mR = wp.tile([128, slots], f32)     # x0 <= W-2 (right valid)
                nc.vector.tensor_scalar(out=mR[:], in0=x0[:],
                                        scalar1=float(W - 1), scalar2=None,
                                        op0=Alu.is_lt)
                ns = wp.tile([128, slots], f32)     # 1 - s
                nc.vector.tensor_scalar(out=ns[:], in0=s[:], scalar1=-1.0,
                                        scalar2=1.0, op0=Alu.mult, op1=Alu.add)
                wR = wp.tile([128, slots], f32)
                nc.vector.tensor_mul(out=wR[:], in0=wx[:], in1=mR[:])
                nc.vector.tensor_mul(out=wR[:], in0=wR[:], in1=ns[:])
                # wT = sy ? wy : 1-wy ;  wB = (sy | y0>=H-1) ? 0 : wy
                one_m_wy = wp.tile([128, slots], f32)
                nc.vector.tensor_scalar(out=one_m_wy[:], in0=wy[:],
                                        scalar1=-1.0, scalar2=1.0,
                                        op0=Alu.mult, op1=Alu.add)
                dy = wp.tile([128, slots], f32)
                nc.vector.tensor_sub(out=dy[:], in0=wy[:], in1=one_m_wy[:])
                wT = wp.tile([128, slots], f32)
                nc.vector.tensor_mul(out=wT[:], in0=sy[:], in1=dy[:])
                nc.vector.tensor_add(out=wT[:], in0=wT[:], in1=one_m_wy[:])
                mB = wp.tile([128, slots], f32)     # y0 <= H-2 (bottom valid)
                nc.vector.tensor_scalar(out=mB[:], in0=y0[:],
                                        scalar1=float(H - 1), scalar2=None,
                                        op0=Alu.is_lt)
                nsy = wp.tile([128, slots], f32)    # 1 - sy
                nc.vector.tensor_scalar(out=nsy[:], in0=sy[:], scalar1=-1.0,
                                        scalar2=1.0, op0=Alu.mult,
                                        op1=Alu.add)
                wBv = wp.tile([128, slots], f32)
                nc.vector.tensor_mul(out=wBv[:], in0=wy[:], in1=mB[:])
                nc.vector.tensor_mul(out=wBv[:], in0=wBv[:], in1=nsy[:])
                # quadrant weights in gather order [T0, B0, T1, B1]
                nc.vector.tensor_mul(out=wq[:, :, 0], in0=wT[:], in1=wL[:])
                nc.vector.tensor_mul(out=wq[:, :, 1], in0=wBv[:], in1=wL[:])
                nc.vector.tensor_mul(out=wq[:, :, 2], in0=wT[:], in1=wR[:])
                nc.vector.tensor_mul(out=wq[:, :, 3], in0=wBv[:], in1=wR[:])

            # ---- gathers + batched blend ----
            # prepare_only + trigger_dma decouples each queue's Q7
            # descriptor generation from its DMA completion: the prep
            # retires after desc-gen, so gen(i+1) overlaps dma(i) within
            # a queue. Tile attributes the gathered tile's write to the
            # prep's DMA-completion tick, so blends still wait correctly.
            dma_sems = ([nc.alloc_semaphore(f"gdma{q}") for q in range(NQ)]
                        if _PREP else None)
            if _PREP:
                # HW sems persist across NEFF runs; zero them before use.
                nums = sorted(s.num for s in dma_sems)
                nc.gpsimd.dma_reset(range(nums[0], nums[-1] + 1))
            out_r = out[:].rearrange("p (s c) -> p s c", c=C)
            groups = [schedule[i:i + _BATCH]
                      for i in range(0, len(schedule), _BATCH)]
            emit_every = max(1, -(-len(groups) // OUT_CHUNKS))
            ci = 0
            off16 = 0
            slot_off = 0
            emitted = 0
            for bi, group in enumerate(groups):
                bs = sum(nj for _, nj in group) // 128
                g = gp.tile([128, bs, ELEM], f32, tag="g", bufs=_GBUFS)
                ti = 0
                for win, nj in group:
                    sj = nj // 128
                    in_ap = bass.AP(
                        table, win * WINDOW * ENTRY,
                        [(ENTRY, min(WINDOW, ECORE - win * WINDOW) + 1),
                         (1, ELEM)])
                    if ci in warm_state:
                        issue_warm(warm_state[ci])
                    if not _WARM:
                        q = ci % NQ
                    elif ci < 4:
                        q = 0          # only q0 warm yet
                    elif ci < 10:
                        q = ci % 2     # q0, q1 warm
                    elif ci < 19:
                        q = ci % 3     # q0..q2 warm
                    else:
                        q = ci % NQ
                    if _PREP:
                        nc.gpsimd.dma_gather(
                            out_ap=g[:, ti:ti + sj, :],
                            in_ap=in_ap,
                            idxs_ap=idx_t[:, off16:off16 + nj // 16],
                            num_idxs=nj,
                            num_idxs_reg=nj,
                            elem_size=ELEM,
                            elem_step=ENTRY,
                            queue_num=q,
                            prepare_only=True,
                            sem=dma_sems[q],
                        )
                        nc.gpsimd.trigger_dma(count=None, queue_num=q)
                    else:
                        nc.gpsimd.dma_gather(
                            out_ap=g[:, ti:ti + sj, :],
                            in_ap=in_ap,
                            idxs_ap=idx_t[:, off16:off16 + nj // 16],
                            num_idxs=nj,
                            num_idxs_reg=nj,
                            elem_size=ELEM,
                            elem_step=ENTRY,
                            queue_num=q,
                        )
                    ci += 1
                    ti += sj
                    off16 += nj // 16
                sl = slice(slot_off, slot_off + bs)
                if _BLEND3:
                    # Alternate PSUM/SBUF operands so no DVE op reads two
                    # SBUF tensors: a 2-SBUF-read op takes the shared SBUF
                    # port pair and locks the Q7 SWDGE descriptor
                    # generators out for its whole duration. A DVE op may
                    # read at most ONE input from PSUM (NCC_IBVF027), so
                    # the reduction alternates its outputs between sides.
                    g4 = g[:, :, :].rearrange("p s (q c) -> p s q c", c=CP)
                    tA = pp.tile([128, bs, 2, C], f32, tag="tA", bufs=1)
                    nc.vector.tensor_tensor(
                        out=tA[:, :, :, :], in0=g4[:, :, 0:2, 0:C],
                        in1=wq[:, sl, 0:2].unsqueeze(3)
                            .to_broadcast([128, bs, 2, C]),
                        op=Alu.mult)
                    tB = gp.tile([128, bs, 2, C], f32, tag="tB", bufs=2)
                    nc.vector.tensor_tensor(
                        out=tB[:, :, :, :], in0=g4[:, :, 2:4, 0:C],
                        in1=wq[:, sl, 2:4].unsqueeze(3)
                            .to_broadcast([128, bs, 2, C]),
                        op=Alu.mult)
                    t2a = gp.tile([128, bs, C], f32, tag="t2a", bufs=2)
                    nc.vector.tensor_tensor(
                        out=t2a[:, :, :], in0=tA[:, :, 0, :],
                        in1=tB[:, :, 0, :], op=Alu.add)
                    t2b = pp.tile([128, bs, C], f32, tag="t2b", bufs=1)
                    nc.vector.tensor_tensor(
                        out=t2b[:, :, :], in0=tA[:, :, 1, :],
                        in1=tB[:, :, 1, :], op=Alu.add)
                    nc.vector.tensor_tensor(
                        out=out_t[:, sl, :], in0=t2b[:, :, :],
                        in1=t2a[:, :, :], op=Alu.add)
                else:
                    tmp = gp.tile([128, bs, C], f32, tag="tmp", bufs=2)
                    nc.vector.tensor_tensor(
                        out=tmp[:, :, :], in0=g[:, :, 0:C],
                        in1=wq[:, sl, 0].unsqueeze(2)
                            .to_broadcast([128, bs, C]),
                        op=Alu.mult)
                    tmp2 = gp.tile([128, bs, C], f32, tag="tmp2", bufs=2)
                    nc.vector.tensor_tensor(
                        out=tmp2[:, :, :], in0=g[:, :, CP:CP + C],
                        in1=wq[:, sl, 1].unsqueeze(2)
                            .to_broadcast([128, bs, C]),
                        op=Alu.mult)
                    nc.vector.tensor_add(out=tmp[:, :, :], in0=tmp[:, :, :],
                                         in1=tmp2[:, :, :])
                    nc.vector.tensor_tensor(
                        out=tmp2[:, :, :], in0=g[:, :, 2 * CP:2 * CP + C],
                        in1=wq[:, sl, 2].unsqueeze(2)
                            .to_broadcast([128, bs, C]),
                        op=Alu.mult)
                    nc.vector.tensor_add(out=tmp[:, :, :], in0=tmp[:, :, :],
                                         in1=tmp2[:, :, :])
                    nc.vector.tensor_tensor(
                        out=tmp2[:, :, :], in0=g[:, :, 3 * CP:3 * CP + C],
                        in1=wq[:, sl, 3].unsqueeze(2)
                            .to_broadcast([128, bs, C]),
                        op=Alu.mult)
                    nc.vector.tensor_add(out=out_t[:, sl, :],
                                         in0=tmp[:, :, :], in1=tmp2[:, :, :])
                slot_off += bs
                if (bi + 1) % emit_every == 0 or bi == len(groups) - 1:
                    nc.sync.dma_start(out=out_r[:, emitted:slot_off, :],
                                      in_=out_t[:, emitted:slot_off, :])
                    emitted = slot_off
    nc.compile()
    return nc


def kernel(values, indices, coors):
    values = np.asarray(values, dtype=np.float32)
    indices = np.asarray(indices, dtype=np.int32)
    coors = np.asarray(coors, dtype=np.float32)
    P = indices.shape[0]

    # ---------- host: banded pixel-pair tables ----------
    v = values.reshape(NIMG, H, W, C)
    px = np.zeros((NIMG, H + 2, W, CP), np.float32)  # +2 pad rows (copy B tail)
    px[:, :H, :, :C] = v
    # copy A bands: rows (2b, 2b+1); copy B bands: rows (2b+1, 2b+2)
    A = px[:, :H].reshape(NIMG, BANDS, 2, W, CP).transpose(0, 1, 3, 2, 4)
    Bc = px[:, 1:H + 1].reshape(NIMG, BANDS, 2, W, CP).transpose(0, 1, 3, 2, 4)
    # unit u = ((img*2 + copy)*2 + half); core = u % 8, local = u // 8
    # per-core table: [18 units, 64 bands, W, 2, CP] -> [ECORE, ENTRY]
    AB = np.stack([A, Bc], axis=1).reshape(NIMG * 2, 2, UNIT_BANDS, W, 2 * CP)
    AB = AB.reshape(NUNITS, ENTRIES_PER_UNIT, ENTRY)

    # ---------- host: route points ----------
    img = (indices[:, 0] * T + indices[:, 1]) * N + indices[:, 2]
    ix = coors[:, 1] - 0.5
    iy = coors[:, 0] - 0.5
    x0 = np.floor(ix).astype(np.int64)
    y0 = np.floor(iy).astype(np.int64)
    sflag = x0 < 0
    xa = x0 + sflag  # in [0, W-1]
    k = np.where(y0 < 0, 0, y0 & 1)  # copy
    band = np.maximum(0, (y0 - k) >> 1)
    half = (band >= UNIT_BANDS).astype(np.int64)
    unit = (img * 2 + k) * 2 + half

    # balance units over cores by point count (greedy, 18 units/core)
    ucnt = np.bincount(unit, minlength=NUNITS)
    load = np.zeros(NCORES, np.int64)
    nun = np.zeros(NCORES, np.int64)
    core_of = np.zeros(NUNITS, np.int64)
    lunit_of = np.zeros(NUNITS, np.int64)
    for u in np.argsort(-ucnt, kind="stable"):
        elig = np.nonzero(nun < UNITS_PER_CORE)[0]
        c = elig[np.argmin(load[elig])]
        core_of[u] = c
        lunit_of[u] = nun[c]
        nun[c] += 1
        load[c] += ucnt[u]

    tables = []
    for c in range(NCORES):
        us = np.nonzero(core_of == c)[0]
        us = us[np.argsort(lunit_of[us])]
        tc_ = np.zeros((ECORE + 2, ENTRY), np.float32)
        tc_[:ECORE] = AB[us].reshape(ECORE, ENTRY)
        tables.append(tc_)

    core = core_of[unit]
    lunit = lunit_of[unit]
    e = (lunit * UNIT_BANDS + (band - half * UNIT_BANDS)) * W + xa
    win = e >> 15

    # per-core sorted orders and per-(core,window) counts
    orders = []
    counts = np.zeros((NCORES, NWIN), np.int64)
    for c in range(NCORES):
        pid = np.nonzero(core == c)[0]
        o = pid[np.argsort(e[pid], kind="stable")]
        orders.append(o)
        cw = np.bincount(win[o], minlength=NWIN)
        counts[c] = cw

    capw = (-(-counts.max(axis=0) // 128) * 128)
    schedule = []
    for w in range(NWIN):
        left = int(capw[w])
        while left > 0:
            nj = min(left, MAX_CALL)
            schedule.append((w, nj))
            left -= nj
    # big calls first: the final calls (and so the last desc-gen, blend
    # and output chunk on the critical tail) are the small leftovers
    schedule = tuple(sorted(schedule, key=lambda t: -t[1]))

    if schedule not in _cache:
        _cache[schedule] = _build_program(schedule)
    nc = _cache[schedule]

    tot_idx = sum(n for _, n in schedule)
    slots = tot_idx // 128

    # stream-position permutation: window-major layout -> schedule order
    woff = np.concatenate([[0], np.cumsum(capw)])[:NWIN]
    perm_src = np.empty(tot_idx, np.int64)
    wcur = {w: 0 for w in range(NWIN)}
    base = 0
    for w, nj in schedule:
        s = wcur[w]
        perm_src[base:base + nj] = woff[w] + s + np.arange(nj)
        wcur[w] = s + nj
        base += nj
    perm_inv = np.empty(tot_idx, np.int64)
    perm_inv[perm_src] = np.arange(tot_idx)

    # ---------- host: per-core idx stream, coors spray ----------
    in_maps = []
    unpack = []  # (order, stream positions of valid points)
    for c in range(NCORES):
        o = orders[c]
        pad_idx = -1 if _NEGPAD else 0  # -1 pads: trimmed by ucode
        idx_stream = np.full(tot_idx, pad_idx, np.int16)
        co_stream = np.full((tot_idx, 2), -1000.5, np.float32)  # pads: w=0
        valid_pos = np.zeros(len(o), np.int64)
        # fill per window: points first, pad (idx -1) after
        coff = np.concatenate([[0], np.cumsum(counts[c])])[:NWIN]
        for w in range(NWIN):
            n = int(counts[c, w])
            if n == 0:
                continue
            pts = o[coff[w]:coff[w] + n]
            pos = woff[w] + np.arange(n)
            idx_stream[pos] = (e[pts] - w * WINDOW).astype(np.int16)
            co_stream[pos] = coors[pts]
            valid_pos[coff[w]:coff[w] + n] = pos
        # window-major layout -> schedule-order stream
        idx_stream = idx_stream[perm_src]
        co_stream = co_stream[perm_src]
        valid_pos = perm_inv[valid_pos]
        # wrap idxs per call: within call block, idx j -> [j%16, j//16]
        blocks = []
        base = 0
        for _, nj in schedule:
            blocks.append(idx_stream[base:base + nj].reshape(nj // 16, 16).T)
            base += nj
        idx_wrapped = np.tile(np.concatenate(blocks, axis=1), (8, 1))
        # spray: stream pos q -> partition q%128, slot q//128
        co_spray = co_stream.reshape(slots, 128, 2).transpose(1, 0, 2)
        in_maps.append({
            "table": tables[c],
            "idxs": idx_wrapped,
            "coors": np.ascontiguousarray(co_spray).reshape(128, slots * 2),
        })
        unpack.append((o, valid_pos))

    global _last_in_maps
    _last_in_maps = in_maps
    from concourse.bass_utils import run_bass_kernel_spmd
    res = run_bass_kernel_spmd(nc, in_maps, list(range(NCORES)))

    out = np.zeros((P, C), np.float32)
    for c in range(NCORES):
        o, valid_pos = unpack[c]
        stream = res.results[c]["out"].reshape(128, slots, C) \
            .transpose(1, 0, 2).reshape(tot_idx, C)
        out[o] = stream[valid_pos]
    return out


# revision 34
# speedup vs baseline: 1.0936x; 1.0936x over previous
"""Trainium2 Bass kernel for GaussianFlowOccRasterizer bilinear point sampling.

values [2,3,6,256,704,17] f32, indices [500000,3] i32, coors [500000,2] f32
-> out [500000,17] f32 (per-point bilinear sample of image flat(b,t,n) at
pixel (coors - 0.5), zero padding outside).

Strategy (8 NeuronCores):
  - Host re-lays values into "banded pixel-pair" tables: copy A holds row
    pairs (2b,2b+1), copy B holds (2b+1,2b+2); each pixel padded 17->32 f32
    so a band entry (band,x) is 256B = [row_top pix | row_bot pix]. One
    512B dma_gather descriptor starting at entry (band,x0) then fetches all
    4 bilinear corners of a point.
  - 144 band-units (36 img x 2 copies x 2 half-images) are dealt
    round-robin to the 8 cores (uniform 207MB table / core). Points are
    routed to the owning core and sorted by 32768-entry windows so the
    int16 dma_gather indices are window-relative (sharding hint: "route
    points to the owning device by flat index").
  - Device: dma_gather spread over all 4 SWDGE queues (disjoint Q7 cpu
    pairs -> 4x parallel descriptor generation), IRAM pre-warmed with
    dummy gathers, 3-op strided DVE blend, chunked output DMA. Trailing
    window-padding indices are negative so the Q7 ucode trims them.
"""
import os

import numpy as np

_WARM = os.environ.get("KERNEL_WARM", "1") == "1"
_BLEND3 = os.environ.get("KERNEL_BLEND3", "1") == "1"
_NEGPAD = os.environ.get("KERNEL_NEGPAD", "0") == "1"  # crashes HW: keep 0
_BATCH = int(os.environ.get("KERNEL_BATCH", "3"))  # gather calls per blend
_GBUFS = int(os.environ.get("KERNEL_GBUFS", "3"))
_PSUM = os.environ.get("KERNEL_PSUM", "1") == "1"  # blend tiles in PSUM
_PREP = os.environ.get("KERNEL_PREP", "0") == "1"  # broken on HW: keep 0

B, T, N, H, W, C = 2, 3, 6, 256, 704, 17
NIMG = B * T * N
NCORES = 8
CP = 32               # padded channels (128 B / pixel)
ELEM = 4 * CP         # gathered f32 per descriptor (2 entries = 4 pixels)
ENTRY = 2 * CP        # f32 per table entry (pixel pair of one x)
BANDS = H // 2        # 128 bands per copy
UNIT_BANDS = BANDS // 2  # 64 bands per unit (half image per copy)
NUNITS = NIMG * 2 * 2    # 144 units
UNITS_PER_CORE = NUNITS // NCORES  # 18
ENTRIES_PER_UNIT = UNIT_BANDS * W  # 45056
ECORE = UNITS_PER_CORE * ENTRIES_PER_UNIT  # 811008 entries per core
WINDOW = 32768
NWIN = -(-ECORE // WINDOW)  # 25
# 1024 idx = 64 descs/SDMA-engine = the single_packet HW ceiling; more crashes
MAX_CALL = int(os.environ.get("KERNEL_MAXCALL", "1024"))
NQ = 4                # SWDGE queues
OUT_CHUNKS = int(os.environ.get("KERNEL_CHUNKS", "8"))

_cache = {}


def _build_program(schedule):
    """schedule: tuple of (window_id, num_idxs) per dma_gather call,
    num_idxs % 128 == 0, <= MAX_CALL. Same program for all cores."""
    import concourse.bacc as bacc
    import concourse.bass as bass
    import concourse.mybir as mybir
    from concourse import library_config
    from concourse.tile import TileContext

    tot_idx = sum(n for _, n in schedule)
    slots = tot_idx // 128

    nc = bacc.Bacc("TRN2", target_bir_lowering=False, debug=False,
                   num_devices=NCORES, num_swdge_queues=NQ)
    table = nc.declare_dram_parameter(
        "table", [ECORE + 2, ENTRY], mybir.dt.float32, isOutput=False)
    idxs = nc.declare_dram_parameter(
        "idxs", [128, tot_idx // 16], mybir.dt.int16, isOutput=False)
    coors = nc.declare_dram_parameter(
        "coors", [128, slots * 2], mybir.dt.float32, isOutput=False)
    out = nc.declare_dram_parameter(
        "out", [128, slots * C], mybir.dt.float32, isOutput=True)

    f32 = mybir.dt.float32
    Alu = mybir.AluOpType

    with TileContext(nc) as tc:
        with tc.tile_pool(name="io", bufs=1) as io, \
             tc.tile_pool(name="gp", bufs=1) as gp, \
             tc.psum_pool(name="pp", bufs=1) as pp:
            nc.gpsimd.load_library(library_config.mlp)

            # inputs
            idx_t = io.tile([128, tot_idx // 16], mybir.dt.int16)
            nc.sync.dma_start(out=idx_t[:], in_=idxs[:])
            co_t = io.tile([128, slots, 2], f32)
            nc.sync.dma_start(
                out=co_t[:, :, :],
                in_=coors[:].rearrange("p (s c) -> p s c", c=2))
            out_t = io.tile([128, slots, C], f32)
            # wq in PSUM: the blend mults then read one SBUF (gathered g)
            # and one PSUM (weights) operand -> no shared-SBUF-port lock.
            wq = (pp if _PSUM else io).tile([128, slots, 4], f32)

            # IRAM warm-up: one dummy gather per queue (each queue runs on
            # its own Q7 cpu pair; first use pays a ~9-13us code load and
            # the loads SERIALIZE on a shared loader). The warms are
            # interleaved with real gathers on already-warm queues (see
            # the ramp in the main loop). memset on gpsimd so warm q0
            # doesn't wait for the DVE preamble.
            warm_state = {}
            if _WARM:
                wi = io.tile([128, 8], mybir.dt.int16)
                nc.gpsimd.memset(wi[:, :], 0)
                warm_ap = bass.AP(table, 0, [(ENTRY, 2), (1, ELEM)])

                def issue_warm(q):
                    wg = io.tile([128, 1, ELEM], f32, name=f"wg{q}",
                                 uniquify=True)
                    nc.gpsimd.dma_gather(
                        out_ap=wg[:, :, :], in_ap=warm_ap, idxs_ap=wi[:, :],
                        num_idxs=128, num_idxs_reg=128, elem_size=ELEM,
                        elem_step=ENTRY, queue_num=q)

                # q3 stays cold: its first real gather absorbs the IRAM
                # load at the same serial stream position the 4th warm
                # would occupy, letting q0-q2 real gens overlap it.
                for q in range(NQ - 1):
                    issue_warm(q)

            # ---- bilinear weights from coors -> wq[128, slots, 4] ----
            with tc.tile_pool(name="wp", bufs=1) as wp:
                def floor_of(src_ap, name):
                    ti = wp.tile([128, slots], mybir.dt.int32, name=f"{name}_i")
                    nc.vector.tensor_copy(out=ti[:], in_=src_ap)
                    tf = wp.tile([128, slots], f32, name=f"{name}_f")
                    nc.vector.tensor_copy(out=tf[:], in_=ti[:])
                    gt = wp.tile([128, slots], f32, name=f"{name}_g")
                    nc.vector.tensor_tensor(out=gt[:], in0=tf[:], in1=src_ap,
                                            op=Alu.is_gt)
                    fl = wp.tile([128, slots], f32, name=f"{name}_fl")
                    nc.vector.tensor_sub(out=fl[:], in0=tf[:], in1=gt[:])
                    return fl

                ix = wp.tile([128, slots], f32)
                nc.vector.tensor_scalar(out=ix[:], in0=co_t[:, :, 1],
                                        scalar1=0.5, scalar2=None,
                                        op0=Alu.subtract)
                iy = wp.tile([128, slots], f32)
                nc.vector.tensor_scalar(out=iy[:], in0=co_t[:, :, 0],
                                        scalar1=0.5, scalar2=None,
                                        op0=Alu.subtract)
                x0 = floor_of(ix[:], "x0")
                y0 = floor_of(iy[:], "y0")
                wx = wp.tile([128, slots], f32)
                nc.vector.tensor_sub(out=wx[:], in0=ix[:], in1=x0[:])
                wy = wp.tile([128, slots], f32)
                nc.vector.tensor_sub(out=wy[:], in0=iy[:], in1=y0[:])

                s = wp.tile([128, slots], f32)      # x0 < 0 (left shift)
                nc.vector.tensor_scalar(out=s[:], in0=x0[:], scalar1=0.0,
                                        scalar2=None, op0=Alu.is_lt)
                sy = wp.tile([128, slots], f32)     # y0 < 0 (top shift)
                nc.vector.tensor_scalar(out=sy[:], in0=y0[:], scalar1=0.0,
                                        scalar2=None, op0=Alu.is_lt)
                # wL = s ? wx : 1-wx ;  wR = (s | x0>=W-1) ? 0 : wx
                one_m_wx = wp.tile([128, slots], f32)
                nc.vector.tensor_scalar(out=one_m_wx[:], in0=wx[:],
                                        scalar1=-1.0, scalar2=1.0,
                                        op0=Alu.mult, op1=Alu.add)
                d = wp.tile([128, slots], f32)
                nc.vector.tensor_sub(out=d[:], in0=wx[:], in1=one_m_wx[:])
                wL = wp.tile([128, slots], f32)
                nc.vector.tensor_mul(out=wL[:], in0=s[:], in1=d[:])
                nc.vector.tensor_add(out=wL[:], in0=wL[:], in1=one_m_wx[:])
                mR = wp.tile([128, slots], f32)     # x0 <= W-2 (right valid)
                nc.vector.tensor_scalar(out=mR[:], in0=x0[:],
                                        scalar1=float(W - 1), scalar2=None,
                                        op0=Alu.is_lt)
                ns = wp.tile([128, slots], f32)     # 1 - s
                nc.vector.tensor_scalar(out=ns[:], in0=s[:], scalar1=-1.0,
                                        scalar2=1.0, op0=Alu.mult, op1=Alu.add)
                wR = wp.tile([128, slots], f32)
                nc.vector.tensor_mul(out=wR[:], in0=wx[:], in1=mR[:])
                nc.vector.tensor_mul(out=wR[:], in0=wR[:], in1=ns[:])
                # wT = sy ? wy : 1-wy ;  wB = (sy | y0>=H-1) ? 0 : wy
                one_m_wy = wp.tile([128, slots], f32)
                nc.vector.tensor_scalar(out=one_m_wy[:], in0=wy[:],
                                        scalar1=-1.0, scalar2=1.0,
                                        op0=Alu.mult, op1=Alu.add)
                dy = wp.tile([128, slots], f32)
                nc.vector.tensor_sub(out=dy[:], in0=wy[:], in1=one_m_wy[:])
                wT = wp.tile([128, slots], f32)
                nc.vector.tensor_mul(out=wT[:], in0=sy[:], in1=dy[:])
                nc.vector.tensor_add(out=wT[:], in0=wT[:], in1=one_m_wy[:])
                mB = wp.tile([128, slots], f32)     # y0 <= H-2 (bottom valid)
                nc.vector.tensor_scalar(out=mB[:], in0=y0[:],
                                        scalar1=float(H - 1), scalar2=None,
                                        op0=Alu.is_lt)
                nsy = wp.tile([128, slots], f32)    # 1 - sy
                nc.vector.tensor_scalar(out=nsy[:], in0=sy[:], scalar1=-1.0,
                                        scalar2=1.0, op0=Alu.mult,
                                        op1=Alu.add)
                wBv = wp.tile([128, slots], f32)
                nc.vector.tensor_mul(out=wBv[:], in0=wy[:], in1=mB[:])
                nc.vector.tensor_mul(out=wBv[:], in0=wBv[:], in1=nsy[:])
                # quadrant weights in gather order [T0, B0, T1, B1]
                nc.vector.tensor_mul(out=wq[:, :, 0], in0=wT[:], in1=wL[:])
                nc.vector.tensor_mul(out=wq[:, :, 1], in0=wBv[:], in1=wL[:])
                nc.vector.tensor_mul(out=wq[:, :, 2], in0=wT[:], in1=wR[:])
                nc.vector.tensor_mul(out=wq[:, :, 3], in0=wBv[:], in1=wR[:])

            # ---- gathers + batched blend ----
            # prepare_only + trigger_dma decouples each queue's Q7
            # descriptor generation from its DMA completion: the prep
            # retires after desc-gen, so gen(i+1) overlaps dma(i) within
            # a queue. Tile attributes the gathered tile's write to the
            # prep's DMA-completion tick, so blends still wait correctly.
            dma_sems = ([nc.alloc_semaphore(f"gdma{q}") for q in range(NQ)]
                        if _PREP else None)
            if _PREP:
                # HW sems persist across NEFF runs; zero them before use.
                nums = sorted(s.num for s in dma_sems)
                nc.gpsimd.dma_reset(range(nums[0], nums[-1] + 1))
            out_r = out[:].rearrange("p (s c) -> p s c", c=C)
            groups = [schedule[i:i + _BATCH]
                      for i in range(0, len(schedule), _BATCH)]
            emit_every = max(1, -(-len(groups) // OUT_CHUNKS))
            ci = 0
            off16 = 0
            slot_off = 0
            emitted = 0
            for bi, group in enumerate(groups):
                bs = sum(nj for _, nj in group) // 128
                g = gp.tile([128, bs, ELEM], f32, tag="g", bufs=_GBUFS)
                ti = 0
                for win, nj in group:
                    sj = nj // 128
                    in_ap = bass.AP(
                        table, win * WINDOW * ENTRY,
                        [(ENTRY, min(WINDOW, ECORE - win * WINDOW) + 1),
                         (1, ELEM)])
                    q = ci % NQ
                    if _PREP:
                        nc.gpsimd.dma_gather(
                            out_ap=g[:, ti:ti + sj, :],
                            in_ap=in_ap,
                            idxs_ap=idx_t[:, off16:off16 + nj // 16],
                            num_idxs=nj,
                            num_idxs_reg=nj,
                            elem_size=ELEM,
                            elem_step=ENTRY,
                            queue_num=q,
                            prepare_only=True,
                            sem=dma_sems[q],
                        )
                        nc.gpsimd.trigger_dma(count=None, queue_num=q)
                    else:
                        nc.gpsimd.dma_gather(
                            out_ap=g[:, ti:ti + sj, :],
                            in_ap=in_ap,
                            idxs_ap=idx_t[:, off16:off16 + nj // 16],
                            num_idxs=nj,
                            num_idxs_reg=nj,
                            elem_size=ELEM,
                            elem_step=ENTRY,
                            queue_num=q,
                        )
                    ci += 1
                    ti += sj
                    off16 += nj // 16
                sl = slice(slot_off, slot_off + bs)
                if _BLEND3:
                    # Alternate PSUM/SBUF operands so no DVE op reads two
                    # SBUF tensors: a 2-SBUF-read op takes the shared SBUF
                    # port pair and locks the Q7 SWDGE descriptor
                    # generators out for its whole duration. A DVE op may
                    # read at most ONE input from PSUM (NCC_IBVF027), so
                    # the reduction alternates its outputs between sides.
                    g4 = g[:, :, :].rearrange("p s (q c) -> p s q c", c=CP)
                    tA = pp.tile([128, bs, 2, C], f32, tag="tA", bufs=1)
                    nc.vector.tensor_tensor(
                        out=tA[:, :, :, :], in0=g4[:, :, 0:2, 0:C],
                        in1=wq[:, sl, 0:2].unsqueeze(3)
                            .to_broadcast([128, bs, 2, C]),
                        op=Alu.mult)
                    tB = gp.tile([128, bs, 2, C], f32, tag="tB", bufs=2)
                    nc.vector.tensor_tensor(
                        out=tB[:, :, :, :], in0=g4[:, :, 2:4, 0:C],
                        in1=wq[:, sl, 2:4].unsqueeze(3)
                            .to_broadcast([128, bs, 2, C]),
                        op=Alu.mult)
                    t2a = gp.tile([128, bs, C], f32, tag="t2a", bufs=2)
                    nc.vector.tensor_tensor(
                        out=t2a[:, :, :], in0=tA[:, :, 0, :],
                        in1=tB[:, :, 0, :], op=Alu.add)
                    t2b = pp.tile([128, bs, C], f32, tag="t2b", bufs=1)
                    nc.vector.tensor_tensor(
                        out=t2b[:, :, :], in0=tA[:, :, 1, :],
                        in1=tB[:, :, 1, :], op=Alu.add)
                    nc.vector.tensor_tensor(
                        out=out_t[:, sl, :], in0=t2b[:, :, :],
                        in1=t2a[:, :, :], op=Alu.add)
                else:
                    tmp = gp.tile([128, bs, C], f32, tag="tmp", bufs=2)
                    nc.vector.tensor_tensor(
                        out=tmp[:, :, :], in0=g[:, :, 0:C],
                        in1=wq[:, sl, 0].unsqueeze(2)
                            .to_broadcast([128, bs, C]),
                        op=Alu.mult)
                    tmp2 = gp.tile([128, bs, C], f32, tag="tmp2", bufs=2)
                    nc.vector.tensor_tensor(
                        out=tmp2[:, :, :], in0=g[:, :, CP:CP + C],
                        in1=wq[:, sl, 1].unsqueeze(2)
                            .to_broadcast([128, bs, C]),
                        op=Alu.mult)
                    nc.vector.tensor_add(out=tmp[:, :, :], in0=tmp[:, :, :],
                                         in1=tmp2[:, :, :])
                    nc.vector.tensor_tensor(
                        out=tmp2[:, :, :], in0=g[:, :, 2 * CP:2 * CP + C],
                        in1=wq[:, sl, 2].unsqueeze(2)
                            .to_broadcast([128, bs, C]),
                        op=Alu.mult)
                    nc.vector.tensor_add(out=tmp[:, :, :], in0=tmp[:, :, :],
                                         in1=tmp2[:, :, :])
                    nc.vector.tensor_tensor(
                        out=tmp2[:, :, :], in0=g[:, :, 3 * CP:3 * CP + C],
                        in1=wq[:, sl, 3].unsqueeze(2)
                            .to_broadcast([128, bs, C]),
                        op=Alu.mult)
                    nc.vector.tensor_add(out=out_t[:, sl, :],
                                         in0=tmp[:, :, :], in1=tmp2[:, :, :])
                slot_off += bs
                if (bi + 1) % emit_every == 0 or bi == len(groups) - 1:
                    nc.sync.dma_start(out=out_r[:, emitted:slot_off, :],
                                      in_=out_t[:, emitted:slot_off, :])
                    emitted = slot_off
    nc.compile()
    return nc


def kernel(values, indices, coors):
    values = np.asarray(values, dtype=np.float32)
    indices = np.asarray(indices, dtype=np.int32)
    coors = np.asarray(coors, dtype=np.float32)
    P = indices.shape[0]

    # ---------- host: banded pixel-pair tables ----------
    v = values.reshape(NIMG, H, W, C)
    px = np.zeros((NIMG, H + 2, W, CP), np.float32)  # +2 pad rows (copy B tail)
    px[:, :H, :, :C] = v
    # copy A bands: rows (2b, 2b+1); copy B bands: rows (2b+1, 2b+2)
    A = px[:, :H].reshape(NIMG, BANDS, 2, W, CP).transpose(0, 1, 3, 2, 4)
    Bc = px[:, 1:H + 1].reshape(NIMG, BANDS, 2, W, CP).transpose(0, 1, 3, 2, 4)
    # unit u = ((img*2 + copy)*2 + half); core = u % 8, local = u // 8
    # per-core table: [18 units, 64 bands, W, 2, CP] -> [ECORE, ENTRY]
    AB = np.stack([A, Bc], axis=1).reshape(NIMG * 2, 2, UNIT_BANDS, W, 2 * CP)
    AB = AB.reshape(NUNITS, ENTRIES_PER_UNIT, ENTRY)

    # ---------- host: route points ----------
    img = (indices[:, 0] * T + indices[:, 1]) * N + indices[:, 2]
    ix = coors[:, 1] - 0.5
    iy = coors[:, 0] - 0.5
    x0 = np.floor(ix).astype(np.int64)
    y0 = np.floor(iy).astype(np.int64)
    sflag = x0 < 0
    xa = x0 + sflag  # in [0, W-1]
    k = np.where(y0 < 0, 0, y0 & 1)  # copy
    band = np.maximum(0, (y0 - k) >> 1)
    half = (band >= UNIT_BANDS).astype(np.int64)
    unit = (img * 2 + k) * 2 + half

    # balance units over cores by point count (greedy, 18 units/core)
    ucnt = np.bincount(unit, minlength=NUNITS)
    load = np.zeros(NCORES, np.int64)
    nun = np.zeros(NCORES, np.int64)
    core_of = np.zeros(NUNITS, np.int64)
    lunit_of = np.zeros(NUNITS, np.int64)
    for u in np.argsort(-ucnt, kind="stable"):
        elig = np.nonzero(nun < UNITS_PER_CORE)[0]
        c = elig[np.argmin(load[elig])]
        core_of[u] = c
        lunit_of[u] = nun[c]
        nun[c] += 1
        load[c] += ucnt[u]

    tables = []
    for c in range(NCORES):
        us = np.nonzero(core_of == c)[0]
        us = us[np.argsort(lunit_of[us])]
        tc_ = np.zeros((ECORE + 2, ENTRY), np.float32)
        tc_[:ECORE] = AB[us].reshape(ECORE, ENTRY)
        tables.append(tc_)

    core = core_of[unit]
    lunit = lunit_of[unit]
    e = (lunit * UNIT_BANDS + (band - half * UNIT_BANDS)) * W + xa
    win = e >> 15

    # per-core sorted orders and per-(core,window) counts
    orders = []
    counts = np.zeros((NCORES, NWIN), np.int64)
    for c in range(NCORES):
        pid = np.nonzero(core == c)[0]
        o = pid[np.argsort(e[pid], kind="stable")]
        orders.append(o)
        cw = np.bincount(win[o], minlength=NWIN)
        counts[c] = cw

    capw = (-(-counts.max(axis=0) // 128) * 128)
    schedule = []
    for w in range(NWIN):
        left = int(capw[w])
        while left > 0:
            nj = min(left, MAX_CALL)
            schedule.append((w, nj))
            left -= nj
    # big calls first: the final calls (and so the last desc-gen, blend
    # and output chunk on the critical tail) are the small leftovers
    schedule = tuple(sorted(schedule, key=lambda t: -t[1]))

    if schedule not in _cache:
        _cache[schedule] = _build_program(schedule)
    nc = _cache[schedule]

    tot_idx = sum(n for _, n in schedule)
    slots = tot_idx // 128

    # stream-position permutation: window-major layout -> schedule order
    woff = np.concatenate([[0], np.cumsum(capw)])[:NWIN]
    perm_src = np.empty(tot_idx, np.int64)
    wcur = {w: 0 for w in range(NWIN)}
    base = 0
    for w, nj in schedule:
        s = wcur[w]
        perm_src[base:base + nj] = woff[w] + s + np.arange(nj)
        wcur[w] = s + nj
        base += nj
    perm_inv = np.empty(tot_idx, np.int64)
    perm_inv[perm_src] = np.arange(tot_idx)

    # ---------- host: per-core idx stream, coors spray ----------
    in_maps = []
    unpack = []  # (order, stream positions of valid points)
    for c in range(NCORES):
        o = orders[c]
        pad_idx = -1 if _NEGPAD else 0  # -1 pads: trimmed by ucode
        idx_stream = np.full(tot_idx, pad_idx, np.int16)
        co_stream = np.full((tot_idx, 2), -1000.5, np.float32)  # pads: w=0
        valid_pos = np.zeros(len(o), np.int64)
        # fill per window: points first, pad (idx -1) after
        coff = np.concatenate([[0], np.cumsum(counts[c])])[:NWIN]
        for w in range(NWIN):
            n = int(counts[c, w])
            if n == 0:
                continue
            pts = o[coff[w]:coff[w] + n]
            pos = woff[w] + np.arange(n)
            idx_stream[pos] = (e[pts] - w * WINDOW).astype(np.int16)
            co_stream[pos] = coors[pts]
            valid_pos[coff[w]:coff[w] + n] = pos
        # window-major layout -> schedule-order stream
        idx_stream = idx_stream[perm_src]
        co_stream = co_stream[perm_src]
        valid_pos = perm_inv[valid_pos]
        # wrap idxs per call: within call block, idx j -> [j%16, j//16]
        blocks = []
        base = 0
        for _, nj in schedule:
            blocks.append(idx_stream[base:base + nj].reshape(nj // 16, 16).T)
            base += nj
        idx_wrapped = np.tile(np.concatenate(blocks, axis=1), (8, 1))
        # spray: stream pos q -> partition q%128, slot q//128
        co_spray = co_stream.reshape(slots, 128, 2).transpose(1, 0, 2)
        in_maps.append({
            "table": tables[c],
            "idxs": idx_wrapped,
            "coors": np.ascontiguousarray(co_spray).reshape(128, slots * 2),
        })
        unpack.append((o, valid_pos))

    global _last_in_maps
    _last_in_maps = in_maps
    from concourse.bass_utils import run_bass_kernel_spmd
    res = run_bass_kernel_spmd(nc, in_maps, list(range(NCORES)))

    out = np.zeros((P, C), np.float32)
    for c in range(NCORES):
        o, valid_pos = unpack[c]
        stream = res.results[c]["out"].reshape(128, slots, C) \
            .transpose(1, 0, 2).reshape(tot_idx, C)
        out[o] = stream[valid_pos]
    return out
